# revision 2
# baseline (speedup 1.0000x reference)
"""Bit-exact bf16-sequential-accumulation Linear (y = bf16_accum_matmul(x, W^T) + b)
for 8 Trainium2 NeuronCores.

The reference rounds to bf16 after EVERY multiply and EVERY accumulate step
(k-order sequential per row), so a PE-array matmul (fp32 PSUM accumulation)
is numerically wrong (~3.7e-2 rel err). Instead this kernel emulates the
exact rounding sequence on the vector engines:

    for k in 0..K-1:   prod = rne16(x[:,k] * wT[k,:]);  acc = rne16(acc + prod)

Data-parallel over the flattened token dim B (16384 rows): each core takes
2048 rows = 16 partition-blocks of 128 rows, grouped into 4 "quads" of 4
blocks so the accumulate runs as one (128, 4096) tensor_tensor per quad.
Per k-step, engines split the work:
  - products (tensor_scalar, bf16 out, 4x DVE mode / ACT activation-Copy
    with per-partition fp32 scale)
  - accumulates (tensor_tensor add, bf16 out, 2x DVE mode / Pool)
x enters only as per-partition scalars (host-pretransposed fp32 xc[p,k,b]);
wT rows are staged to partition 0 by DMA and partition_broadcast to 128
partitions chunk-by-chunk. All ops verified bit-exact vs XLA-CPU semantics.
"""

import numpy as np
import ml_dtypes
from contextlib import ExitStack

import concourse.bacc as bacc
import concourse.mybir as mybir
from concourse import tile
from concourse.bass_utils import run_bass_kernel_spmd

BF16 = ml_dtypes.bfloat16
DT = mybir.dt

P = 128          # SBUF partitions
NBLK = 16        # row blocks per core -> 2048 rows/core
NQUAD = 4
N = 1024         # output features
K = 1024         # contraction length
KC = 8           # k's per broadcast chunk
NCORES = 8
ROWS_PER_CORE = NBLK * P

# per-block TS engine ('v'=DVE, 'a'=ACT) and per-quad TT engine ('v'=DVE, 'p'=Pool).
# Measured on HW (calibrated vs a stub kernel in the same session): this split
# gives ~11.3ms/core device time vs 15.0ms all-DVE; Pool TT and larger ACT
# shares measured worse (cross-engine sync dominates).
ASSIGN_TS = ["v"] * 6 + ["a"] * 10
ASSIGN_TT = ["v", "v", "v", "v"]


def _build(n_cores: int = NCORES):
    nc = bacc.Bacc("TRN2", target_bir_lowering=False, debug=False, num_devices=n_cores)
    xc = nc.dram_tensor("xc", [P, K, NBLK], DT.float32, kind="ExternalInput")
    wt = nc.dram_tensor("wt", [K, N], DT.bfloat16, kind="ExternalInput")
    bias = nc.dram_tensor("bias", [1, N], DT.bfloat16, kind="ExternalInput")
    y = nc.dram_tensor("y", [ROWS_PER_CORE, N], DT.bfloat16, kind="ExternalOutput")

    nkc = K // KC
    with tile.TileContext(nc) as tc, ExitStack() as ctx:
        const_pool = ctx.enter_context(tc.tile_pool(name="const", bufs=1))
        stage_pool = ctx.enter_context(tc.tile_pool(name="stage", bufs=2))
        wb_pool = ctx.enter_context(tc.tile_pool(name="wb", bufs=2))
        xc_pool = ctx.enter_context(tc.tile_pool(name="xcp", bufs=3))
        prod_pools = [
            ctx.enter_context(tc.tile_pool(name=f"prod{q}", bufs=2)) for q in range(NQUAD)
        ]

        bias_sb = const_pool.tile([1, N], DT.bfloat16, tag="biasrow")
        nc.sync.dma_start(bias_sb[:], bias[:])

        accs = []
        for q in range(NQUAD):
            a = const_pool.tile([P, 4 * N], DT.bfloat16, tag=f"acc{q}")
            nc.gpsimd.memset(a[:], 0.0)
            accs.append(a)

        for kc in range(nkc):
            xt = xc_pool.tile([P, KC * NBLK], DT.float32, tag="xc")
            nc.sync.dma_start(xt[:], xc[:, kc * KC : (kc + 1) * KC, :])

            st = stage_pool.tile([1, KC * N], DT.bfloat16, tag="stage")
            nc.sync.dma_start(
                st[:],
                wt[kc * KC : (kc + 1) * KC, :].rearrange("(o a) b -> o (a b)", o=1),
            )
            wbt = wb_pool.tile([P, KC * N], DT.bfloat16, tag="wb")
            nc.gpsimd.partition_broadcast(wbt[:], st[0:1, :])

            for j in range(KC):
                wslice = wbt[:, j * N : (j + 1) * N]
                prods = []
                for q in range(NQUAD):
                    pq = prod_pools[q].tile([P, 4 * N], DT.bfloat16, tag=f"prod{q}")
                    prods.append(pq)
                    for i in range(4):
                        b = 4 * q + i
                        xs = xt[:, j * NBLK + b : j * NBLK + b + 1]
                        if ASSIGN_TS[b] == "v":
                            nc.vector.tensor_scalar_mul(
                                pq[:, i * N : (i + 1) * N], wslice, xs
                            )
                        else:
                            nc.scalar.mul(pq[:, i * N : (i + 1) * N], wslice, xs)
                for q in range(NQUAD):
                    eng = {"v": nc.vector, "p": nc.gpsimd}[ASSIGN_TT[q]]
                    eng.tensor_tensor(
                        accs[q][:], accs[q][:], prods[q][:], mybir.AluOpType.add
                    )

        bias_bc = const_pool.tile([P, N], DT.bfloat16, tag="biasbc")
        nc.gpsimd.partition_broadcast(bias_bc[:], bias_sb[0:1, :])
        for q in range(NQUAD):
            for i in range(4):
                b = 4 * q + i
                sl = accs[q][:, i * N : (i + 1) * N]
                nc.vector.tensor_tensor(sl, sl, bias_bc[:], mybir.AluOpType.add)
                nc.sync.dma_start(y[b * P : (b + 1) * P, :], sl)

    nc.compile()
    return nc


_NC_CACHE = {}


def _get_nc(n_cores: int = NCORES):
    if n_cores not in _NC_CACHE:
        _NC_CACHE[n_cores] = _build(n_cores)
    return _NC_CACHE[n_cores]


def _host_prep_core(x2d_shard: np.ndarray, wt: np.ndarray, bias2d: np.ndarray):
    xc = (
        x2d_shard.astype(np.float32)
        .reshape(NBLK, P, K)
        .transpose(1, 2, 0)
        .copy()
    )  # (128, K, 16): xc[p, k, b] = x2d_shard[b*128 + p, k]
    return dict(xc=xc, wt=wt, bias=bias2d)


def _make_in_maps(inputs: dict) -> list:
    x = np.asarray(inputs["x"])
    x2d = x.reshape(-1, K)
    wt = np.ascontiguousarray(np.asarray(inputs["weight"]).astype(BF16).T)
    bias2d = np.asarray(inputs["bias"]).astype(BF16).reshape(1, N)
    return [
        _host_prep_core(x2d[c * ROWS_PER_CORE : (c + 1) * ROWS_PER_CORE], wt, bias2d)
        for c in range(NCORES)
    ]


def kernel(x: np.ndarray, weight: np.ndarray, bias: np.ndarray) -> np.ndarray:
    x = np.asarray(x)
    orig_shape = x.shape[:-1]
    x2d = x.reshape(-1, K)
    assert x2d.shape[0] == NCORES * ROWS_PER_CORE, x2d.shape

    wt = np.ascontiguousarray(np.asarray(weight).astype(BF16).T)  # (K, N) = wT
    bias2d = np.asarray(bias).astype(BF16).reshape(1, N)

    nc = _get_nc(NCORES)
    in_maps = [
        _host_prep_core(x2d[c * ROWS_PER_CORE : (c + 1) * ROWS_PER_CORE], wt, bias2d)
        for c in range(NCORES)
    ]
    res = run_bass_kernel_spmd(nc, in_maps, core_ids=list(range(NCORES)))
    y = np.concatenate([res.results[c]["y"] for c in range(NCORES)], axis=0)
    return y.reshape(*orig_shape, N).astype(BF16)



# revision 3
# speedup vs baseline: 1.2163x; 1.2163x over previous
"""Bit-exact bf16-sequential-accumulation Linear (y = bf16_accum_matmul(x, W^T) + b)
for 8 Trainium2 NeuronCores.

The reference rounds to bf16 after EVERY multiply and EVERY accumulate step
(k-order sequential per row), so a PE-array matmul (fp32 PSUM accumulation)
is numerically wrong (~3.7e-2 rel err). Instead this kernel emulates the
exact rounding sequence on the vector engines:

    for k in 0..K-1:   prod = rne16(x[:,k] * wT[k,:]);  acc = rne16(acc + prod)

Data-parallel over the flattened token dim B (16384 rows): each core takes
2048 rows = 16 partition-blocks of 128 rows, grouped into 4 "quads" of 4
blocks so the accumulate runs as one (128, 4096) tensor_tensor per quad.
Per k-step, engines split the work:
  - products (tensor_scalar, bf16 out, 4x DVE mode / ACT activation-Copy
    with per-partition fp32 scale)
  - accumulates (tensor_tensor add, bf16 out, 2x DVE mode / Pool)
x enters only as per-partition scalars (host-pretransposed fp32 xc[p,k,b]);
wT rows are staged to partition 0 by DMA and partition_broadcast to 128
partitions chunk-by-chunk. All ops verified bit-exact vs XLA-CPU semantics.

Optimization notes from a later session (HW-measured, see probe_ops.py /
time_compare.py; baseline re-measured 11.43ms, stub-calibrated min-wall):
- Per-1024el-block engine costs (back-to-back): DVE TT add 520ns (2x),
  DVE TS mul 338ns (4x), ACT mul 857ns, Pool TT add 2312ns,
  DVE scalar_tensor_tensor 1839ns (no perf mode - useless).
- neuronxcc REJECTS TensorScalarPtr on Pool ([NCC_IXCG966]), so no fused
  multiply-accumulate on Pool; Pool can only TT-add at ~2.3ns/el.
- Numerics: skipping the bf16 product pre-round (fp32 product, bf16-rounded
  accumulate) gives rel err 1.60e-2 vs the bit-exact oracle (gate 2e-2);
  any chunked accumulation (bf16 round every 2+ steps) fails the gate.
- The w-broadcast (2MB per 8-k chunk) costs ~90-113us/chunk on ANY single
  mover: a stride-0-source DMA serializes on one HW-DGE queue (~18.7GB/s),
  and a full Pool partition_broadcast costs about the same. This kernel
  hides it only because Pool is otherwise idle (~89us/chunk compute).
  Rebalanced variants that load Pool with TT adds (kernel_v3.py, LP-optimal
  a=3 DVE / b=10 ACT / e=3 Pool -> theoretical 9.0ms) measured 14.9ms
  (DMA-broadcast gated) and 21.7ms (Pool bcast+TT serialized). A multi-mover
  split (SP-HWDGE + ACT-HWDGE dma_start + 16-channel pool broadcast) failed
  neuronxcc codegen in combination, though nc.scalar.dma_start with a
  broadcast_to source compiles in isolation; offset-base partition_broadcast
  (wbt[112:128]) is the remaining suspect. That split is the most promising
  path below ~11ms if debugged.
"""

import numpy as np
import ml_dtypes
from contextlib import ExitStack

import concourse.bacc as bacc
import concourse.mybir as mybir
from concourse import tile
from concourse.bass_utils import run_bass_kernel_spmd

BF16 = ml_dtypes.bfloat16
DT = mybir.dt

P = 128          # SBUF partitions
NBLK = 16        # row blocks per core -> 2048 rows/core
NQUAD = 4
N = 1024         # output features
K = 1024         # contraction length
KC = 8           # k's per broadcast chunk
NCORES = 8
ROWS_PER_CORE = NBLK * P

# per-block TS engine ('v'=DVE, 'a'=ACT) and per-quad TT engine ('v'=DVE, 'p'=Pool).
# Measured on HW (calibrated vs a stub kernel in the same session): this split
# gives ~11.3ms/core device time vs 15.0ms all-DVE; Pool TT and larger ACT
# shares measured worse (cross-engine sync dominates).
ASSIGN_TS = ["v"] * 6 + ["a"] * 10
ASSIGN_TT = ["v", "v", "v", "v"]


def _build(n_cores: int = NCORES):
    nc = bacc.Bacc("TRN2", target_bir_lowering=False, debug=False, num_devices=n_cores)
    xc = nc.dram_tensor("xc", [P, K, NBLK], DT.float32, kind="ExternalInput")
    wt = nc.dram_tensor("wt", [K, N], DT.bfloat16, kind="ExternalInput")
    bias = nc.dram_tensor("bias", [1, N], DT.bfloat16, kind="ExternalInput")
    y = nc.dram_tensor("y", [ROWS_PER_CORE, N], DT.bfloat16, kind="ExternalOutput")

    nkc = K // KC
    with tile.TileContext(nc) as tc, ExitStack() as ctx:
        const_pool = ctx.enter_context(tc.tile_pool(name="const", bufs=1))
        stage_pool = ctx.enter_context(tc.tile_pool(name="stage", bufs=2))
        wb_pool = ctx.enter_context(tc.tile_pool(name="wb", bufs=2))
        xc_pool = ctx.enter_context(tc.tile_pool(name="xcp", bufs=3))
        prod_pools = [
            ctx.enter_context(tc.tile_pool(name=f"prod{q}", bufs=2)) for q in range(NQUAD)
        ]

        bias_sb = const_pool.tile([1, N], DT.bfloat16, tag="biasrow")
        nc.sync.dma_start(bias_sb[:], bias[:])

        accs = []
        for q in range(NQUAD):
            a = const_pool.tile([P, 4 * N], DT.bfloat16, tag=f"acc{q}")
            nc.gpsimd.memset(a[:], 0.0)
            accs.append(a)

        for kc in range(nkc):
            xt = xc_pool.tile([P, KC * NBLK], DT.float32, tag="xc")
            nc.sync.dma_start(xt[:], xc[:, kc * KC : (kc + 1) * KC, :])

            st = stage_pool.tile([1, KC * N], DT.bfloat16, tag="stage")
            nc.sync.dma_start(
                st[:],
                wt[kc * KC : (kc + 1) * KC, :].rearrange("(o a) b -> o (a b)", o=1),
            )
            wbt = wb_pool.tile([P, KC * N], DT.bfloat16, tag="wb")
            nc.gpsimd.partition_broadcast(wbt[:], st[0:1, :])

            for j in range(KC):
                wslice = wbt[:, j * N : (j + 1) * N]
                prods = []
                for q in range(NQUAD):
                    pq = prod_pools[q].tile([P, 4 * N], DT.bfloat16, tag=f"prod{q}")
                    prods.append(pq)
                    for i in range(4):
                        b = 4 * q + i
                        xs = xt[:, j * NBLK + b : j * NBLK + b + 1]
                        if ASSIGN_TS[b] == "v":
                            nc.vector.tensor_scalar_mul(
                                pq[:, i * N : (i + 1) * N], wslice, xs
                            )
                        else:
                            nc.scalar.mul(pq[:, i * N : (i + 1) * N], wslice, xs)
                for q in range(NQUAD):
                    eng = {"v": nc.vector, "p": nc.gpsimd}[ASSIGN_TT[q]]
                    eng.tensor_tensor(
                        accs[q][:], accs[q][:], prods[q][:], mybir.AluOpType.add
                    )

        bias_bc = const_pool.tile([P, N], DT.bfloat16, tag="biasbc")
        nc.gpsimd.partition_broadcast(bias_bc[:], bias_sb[0:1, :])
        for q in range(NQUAD):
            for i in range(4):
                b = 4 * q + i
                sl = accs[q][:, i * N : (i + 1) * N]
                nc.vector.tensor_tensor(sl, sl, bias_bc[:], mybir.AluOpType.add)
                nc.sync.dma_start(y[b * P : (b + 1) * P, :], sl)

    nc.compile()
    return nc


_NC_CACHE = {}


def _get_nc(n_cores: int = NCORES):
    if n_cores not in _NC_CACHE:
        _NC_CACHE[n_cores] = _build(n_cores)
    return _NC_CACHE[n_cores]


def _host_prep_core(x2d_shard: np.ndarray, wt: np.ndarray, bias2d: np.ndarray):
    xc = (
        x2d_shard.astype(np.float32)
        .reshape(NBLK, P, K)
        .transpose(1, 2, 0)
        .copy()
    )  # (128, K, 16): xc[p, k, b] = x2d_shard[b*128 + p, k]
    return dict(xc=xc, wt=wt, bias=bias2d)


def _make_in_maps(inputs: dict) -> list:
    x = np.asarray(inputs["x"])
    x2d = x.reshape(-1, K)
    wt = np.ascontiguousarray(np.asarray(inputs["weight"]).astype(BF16).T)
    bias2d = np.asarray(inputs["bias"]).astype(BF16).reshape(1, N)
    return [
        _host_prep_core(x2d[c * ROWS_PER_CORE : (c + 1) * ROWS_PER_CORE], wt, bias2d)
        for c in range(NCORES)
    ]


def kernel(x: np.ndarray, weight: np.ndarray, bias: np.ndarray) -> np.ndarray:
    x = np.asarray(x)
    orig_shape = x.shape[:-1]
    x2d = x.reshape(-1, K)
    assert x2d.shape[0] == NCORES * ROWS_PER_CORE, x2d.shape

    wt = np.ascontiguousarray(np.asarray(weight).astype(BF16).T)  # (K, N) = wT
    bias2d = np.asarray(bias).astype(BF16).reshape(1, N)

    nc = _get_nc(NCORES)
    in_maps = [
        _host_prep_core(x2d[c * ROWS_PER_CORE : (c + 1) * ROWS_PER_CORE], wt, bias2d)
        for c in range(NCORES)
    ]
    res = run_bass_kernel_spmd(nc, in_maps, core_ids=list(range(NCORES)))
    y = np.concatenate([res.results[c]["y"] for c in range(NCORES)], axis=0)
    return y.reshape(*orig_shape, N).astype(BF16)



# revision 4
# speedup vs baseline: 1.4931x; 1.2276x over previous
"""Bit-exact bf16-sequential-accumulation Linear (y = bf16_accum_matmul(x, W^T) + b)
for 8 Trainium2 NeuronCores.

The reference rounds to bf16 after EVERY multiply and EVERY accumulate step
(k-order sequential per row), so a PE-array matmul (fp32 PSUM accumulation)
is numerically wrong (~3.7e-2 rel err). Instead this kernel emulates the
exact rounding sequence on the vector engines:

    for k in 0..K-1:   prod = rne16(x[:,k] * wT[k,:]);  acc = rne16(acc + prod)

Data-parallel over the flattened token dim B (16384 rows): each core takes
2048 rows = 16 partition-blocks of 128 rows, grouped into 4 "quads" of 4
blocks so the accumulate runs as one (128, 4096) tensor_tensor per quad.
Per k-step, engines split the work:
  - products (tensor_scalar, bf16 out, 4x DVE mode / ACT activation-Copy
    with per-partition fp32 scale)
  - accumulates (tensor_tensor add, bf16 out, 2x DVE mode / Pool)
x enters only as per-partition scalars (host-pretransposed fp32 xc[p,k,b]);
wT rows are staged to partition 0 by DMA and partition_broadcast to 128
partitions chunk-by-chunk. All ops verified bit-exact vs XLA-CPU semantics.

Optimization notes from a later session (HW-measured, see probe_ops.py /
time_compare.py; baseline re-measured 11.43ms, stub-calibrated min-wall):
- Per-1024el-block engine costs (back-to-back): DVE TT add 520ns (2x),
  DVE TS mul 338ns (4x), ACT mul 857ns, Pool TT add 2312ns,
  DVE scalar_tensor_tensor 1839ns (no perf mode - useless).
- neuronxcc REJECTS TensorScalarPtr on Pool ([NCC_IXCG966]), so no fused
  multiply-accumulate on Pool; Pool can only TT-add at ~2.3ns/el.
- Numerics: skipping the bf16 product pre-round (fp32 product, bf16-rounded
  accumulate) gives rel err 1.60e-2 vs the bit-exact oracle (gate 2e-2);
  any chunked accumulation (bf16 round every 2+ steps) fails the gate.
- The w-broadcast (2MB per 8-k chunk) costs ~90-113us/chunk on ANY single
  mover: a stride-0-source DMA serializes on one HW-DGE queue (~18.7GB/s),
  and a full Pool partition_broadcast costs about the same. This kernel
  hides it only because Pool is otherwise idle (~89us/chunk compute).
  Rebalanced variants that load Pool with TT adds (kernel_v3.py, LP-optimal
  a=3 DVE / b=10 ACT / e=3 Pool -> theoretical 9.0ms) measured 14.9ms
  (DMA-broadcast gated) and 21.7ms (Pool bcast+TT serialized).
- Multi-mover broadcast splits BY PARTITION RANGE do NOT help (measured):
  SP-DMA 64 + ACT-DMA 64 -> 17.6ms (ACT-issued DMA stalls the ACT compute
  stream, and the two HWDGE rings share bandwidth); Pool-bcast 64ch +
  SP-DMA 64 -> 17.9ms (partition_broadcast cost is FREE-SIZE-bound, not
  channel-bound: 64 channels cost the same ~90-100us/chunk as 128).
- NEXT STEP (designed, unvalidated): split the broadcast along the FREE dim
  instead: Pool partition_broadcast(wbt[:, 0:KC*N/2], st) (~45us/chunk) in
  parallel with nc.sync.dma_start(wbt[:, KC*N/2:], src2.broadcast_to(
  [128, KC*N/2])) (~53us/chunk, SP-issued only). Both fit under the
  rebalanced compute period (~76us/chunk with e=1 Pool TT block, a=4 DVE /
  b=11 ACT products, merged 15-block DVE TT) -> predicted ~9.7ms. All
  component patterns compile and are HW-verified individually.
"""

import numpy as np
import ml_dtypes
from contextlib import ExitStack

import concourse.bacc as bacc
import concourse.mybir as mybir
from concourse import tile
from concourse.bass_utils import run_bass_kernel_spmd

BF16 = ml_dtypes.bfloat16
DT = mybir.dt

P = 128          # SBUF partitions
NBLK = 16        # row blocks per core -> 2048 rows/core
NQUAD = 4
N = 1024         # output features
K = 1024         # contraction length
KC = 8           # k's per broadcast chunk
NCORES = 8
ROWS_PER_CORE = NBLK * P

# per-block TS engine ('v'=DVE, 'a'=ACT) and per-quad TT engine ('v'=DVE, 'p'=Pool).
# Measured on HW (calibrated vs a stub kernel in the same session): this split
# gives ~11.3ms/core device time vs 15.0ms all-DVE; Pool TT and larger ACT
# shares measured worse (cross-engine sync dominates).
ASSIGN_TS = ["v"] * 6 + ["a"] * 10
ASSIGN_TT = ["v", "v", "v", "v"]


def _build(n_cores: int = NCORES):
    nc = bacc.Bacc("TRN2", target_bir_lowering=False, debug=False, num_devices=n_cores)
    xc = nc.dram_tensor("xc", [P, K, NBLK], DT.float32, kind="ExternalInput")
    wt = nc.dram_tensor("wt", [K, N], DT.bfloat16, kind="ExternalInput")
    bias = nc.dram_tensor("bias", [1, N], DT.bfloat16, kind="ExternalInput")
    y = nc.dram_tensor("y", [ROWS_PER_CORE, N], DT.bfloat16, kind="ExternalOutput")

    nkc = K // KC
    with tile.TileContext(nc) as tc, ExitStack() as ctx:
        const_pool = ctx.enter_context(tc.tile_pool(name="const", bufs=1))
        stage_pool = ctx.enter_context(tc.tile_pool(name="stage", bufs=2))
        wb_pool = ctx.enter_context(tc.tile_pool(name="wb", bufs=2))
        xc_pool = ctx.enter_context(tc.tile_pool(name="xcp", bufs=3))
        prod_pools = [
            ctx.enter_context(tc.tile_pool(name=f"prod{q}", bufs=2)) for q in range(NQUAD)
        ]

        bias_sb = const_pool.tile([1, N], DT.bfloat16, tag="biasrow")
        nc.sync.dma_start(bias_sb[:], bias[:])

        accs = []
        for q in range(NQUAD):
            a = const_pool.tile([P, 4 * N], DT.bfloat16, tag=f"acc{q}")
            nc.gpsimd.memset(a[:], 0.0)
            accs.append(a)

        for kc in range(nkc):
            xt = xc_pool.tile([P, KC * NBLK], DT.float32, tag="xc")
            nc.sync.dma_start(xt[:], xc[:, kc * KC : (kc + 1) * KC, :])

            st = stage_pool.tile([1, KC * N], DT.bfloat16, tag="stage")
            nc.sync.dma_start(
                st[:],
                wt[kc * KC : (kc + 1) * KC, :].rearrange("(o a) b -> o (a b)", o=1),
            )
            wbt = wb_pool.tile([P, KC * N], DT.bfloat16, tag="wb")
            nc.gpsimd.partition_broadcast(wbt[:], st[0:1, :])

            for j in range(KC):
                wslice = wbt[:, j * N : (j + 1) * N]
                prods = []
                for q in range(NQUAD):
                    pq = prod_pools[q].tile([P, 4 * N], DT.bfloat16, tag=f"prod{q}")
                    prods.append(pq)
                    for i in range(4):
                        b = 4 * q + i
                        xs = xt[:, j * NBLK + b : j * NBLK + b + 1]
                        if ASSIGN_TS[b] == "v":
                            nc.vector.tensor_scalar_mul(
                                pq[:, i * N : (i + 1) * N], wslice, xs
                            )
                        else:
                            nc.scalar.mul(pq[:, i * N : (i + 1) * N], wslice, xs)
                for q in range(NQUAD):
                    eng = {"v": nc.vector, "p": nc.gpsimd}[ASSIGN_TT[q]]
                    eng.tensor_tensor(
                        accs[q][:], accs[q][:], prods[q][:], mybir.AluOpType.add
                    )

        bias_bc = const_pool.tile([P, N], DT.bfloat16, tag="biasbc")
        nc.gpsimd.partition_broadcast(bias_bc[:], bias_sb[0:1, :])
        for q in range(NQUAD):
            for i in range(4):
                b = 4 * q + i
                sl = accs[q][:, i * N : (i + 1) * N]
                nc.vector.tensor_tensor(sl, sl, bias_bc[:], mybir.AluOpType.add)
                nc.sync.dma_start(y[b * P : (b + 1) * P, :], sl)

    nc.compile()
    return nc


_NC_CACHE = {}


def _get_nc(n_cores: int = NCORES):
    if n_cores not in _NC_CACHE:
        _NC_CACHE[n_cores] = _build(n_cores)
    return _NC_CACHE[n_cores]


def _host_prep_core(x2d_shard: np.ndarray, wt: np.ndarray, bias2d: np.ndarray):
    xc = (
        x2d_shard.astype(np.float32)
        .reshape(NBLK, P, K)
        .transpose(1, 2, 0)
        .copy()
    )  # (128, K, 16): xc[p, k, b] = x2d_shard[b*128 + p, k]
    return dict(xc=xc, wt=wt, bias=bias2d)


def _make_in_maps(inputs: dict) -> list:
    x = np.asarray(inputs["x"])
    x2d = x.reshape(-1, K)
    wt = np.ascontiguousarray(np.asarray(inputs["weight"]).astype(BF16).T)
    bias2d = np.asarray(inputs["bias"]).astype(BF16).reshape(1, N)
    return [
        _host_prep_core(x2d[c * ROWS_PER_CORE : (c + 1) * ROWS_PER_CORE], wt, bias2d)
        for c in range(NCORES)
    ]


def kernel(x: np.ndarray, weight: np.ndarray, bias: np.ndarray) -> np.ndarray:
    x = np.asarray(x)
    orig_shape = x.shape[:-1]
    x2d = x.reshape(-1, K)
    assert x2d.shape[0] == NCORES * ROWS_PER_CORE, x2d.shape

    wt = np.ascontiguousarray(np.asarray(weight).astype(BF16).T)  # (K, N) = wT
    bias2d = np.asarray(bias).astype(BF16).reshape(1, N)

    nc = _get_nc(NCORES)
    in_maps = [
        _host_prep_core(x2d[c * ROWS_PER_CORE : (c + 1) * ROWS_PER_CORE], wt, bias2d)
        for c in range(NCORES)
    ]
    res = run_bass_kernel_spmd(nc, in_maps, core_ids=list(range(NCORES)))
    y = np.concatenate([res.results[c]["y"] for c in range(NCORES)], axis=0)
    return y.reshape(*orig_shape, N).astype(BF16)

